# revision 28
# baseline (speedup 1.0000x reference)
"""Trainium2 Bass kernel for an AttentionBlock (LN -> QKV -> attn -> out-proj + residual).

Shapes (hardcoded per problem spec): B=8, L=1024, C=1024, H=8 heads.
The reference uses a raw row-major reshape (torch-style .view) of q/k/v from
[B, L, C] to [B*H, L, C/H]; with L=1024, C=1024, H=8 this makes each
"attention head" operate on a contiguous 128-sequence-row block of the
[L, C] matrix, reinterpreted as [1024, 128].

Sharding: pure data-parallel over batch, one batch element per NeuronCore
(8 cores). No collectives.

Perf design: the large contractions (QKV projection, attn*V, softmax sum,
out projection) run as fp8e4m3 matmuls in DoubleRowSwInterleave perf mode
(two 128-deep k-subtiles per instruction; ~2x bf16 FLOP rate). The
stationary operand must be software-interleaved: per partition the free
dim holds [A_127, B_127, ..., A_0, B_0] (A/B = the two k-subtiles'
weights per output column, columns reversed). Host weights arrive
pre-interleaved; device-produced stationaries (xnT, v, attnT) are written
in that layout via negative-stride APs in their epilogues. Weights are
pre-scaled by 32 so fp8 stays in normal range; the 1/32 is folded into
the bias / softmax-normalization epilogues. Scores stay bf16.

Phase order keeps the PE dense (DVFS ramps with utilization):
LN+transpose -> QK proj (interleaved xnT copies overlap) -> V proj ->
per-head [scores -> exp -> sums -> attnV -> out-proj -> residual].
"""

import math
from contextlib import ExitStack

import ml_dtypes
import numpy as np

import concourse.bass as bass
import concourse.bacc as bacc
import concourse.tile as tile
from concourse import mybir
from concourse import bass_utils
from concourse.masks import make_identity

L = 1024
C = 1024
H = 8          # heads; also number of 128-row l-tiles (head h <-> l-tile h)
CH = 128       # head dim
NT = 8         # l tiles (128 rows each)
NG = 8         # c groups (128 cols each)
EPS = 1e-5
S2 = 1.0 / math.sqrt(CH)   # combined q&k scale: (ch^-0.25)^2
WS = 32.0                  # host-side fp8 weight pre-scale
WSI = 1.0 / WS

f32 = mybir.dt.float32
bf16 = mybir.dt.bfloat16
fp8 = mybir.dt.float8e4
u8 = mybir.dt.uint8
AF = mybir.ActivationFunctionType
ALU = mybir.AluOpType
DRI = mybir.MatmulPerfMode.DoubleRowSwInterleave


def _bcast_ap(ap, p=128):
    """View a 1-D DRAM vector as [p, n] with a step-0 partition dim."""
    return bass.AP(tensor=ap.tensor, offset=ap.offset, ap=[[0, p]] + list(ap.ap))


def _ilv_dst(base, extra_offset, outer):
    """Interleaved stationary-layout destination AP: dims (*outer, s=2, j=128)
    with strides (*outer, +1, -2) from offset+254 - writes buf[2j+s] with the
    column index reversed, the layout DoubleRowSwInterleave ldweights wants."""
    return bass.AP(tensor=base.tensor, offset=base.offset + extra_offset + 254,
                   ap=[list(base.ap[0])] + outer + [[1, 2], [-2, 128]])


def _emit(nc, apply_affine: bool):
    x_d = nc.dram_tensor("x", [L, C], f32, kind="ExternalInput").ap()
    # fp8 weights travel as uint8 (XLA on TRN2 rejects fp8 parameter dtypes)
    wqk8_d = nc.dram_tensor("wqk8i", [128, 4, 16, 256], u8, kind="ExternalInput").ap()
    wv8_d = nc.dram_tensor("wv8", [128, NG, C], u8, kind="ExternalInput").ap()
    wout8_d = nc.dram_tensor("wout8", [128, NG, C], u8, kind="ExternalInput").ap()
    bqk_d = nc.dram_tensor("b_qk", [128, 16], f32, kind="ExternalInput").ap()
    bv32_d = nc.dram_tensor("b_v32", [C], f32, kind="ExternalInput").ap()
    bout_d = nc.dram_tensor("b_out", [C], f32, kind="ExternalInput").ap()
    if apply_affine:
        g_d = nc.dram_tensor("ln_g", [C], f32, kind="ExternalInput").ap()
        b_d = nc.dram_tensor("ln_b", [C], f32, kind="ExternalInput").ap()
    out_d = nc.dram_tensor("out", [L, C], f32, kind="ExternalOutput").ap()

    with nc.allow_low_precision(reason="bf16/fp8 compute by design"), \
         tile.TileContext(nc) as tc, ExitStack() as ctx:
        const = ctx.enter_context(tc.tile_pool(name="const", bufs=1, side="left"))
        ident = const.tile([128, 128], bf16)
        make_identity(nc, ident)
        # dual-fp8 ldweights wants the full 128 stationary columns -> 128
        # replicated ones columns (psum rows 1..127 hold sum copies, harmless)
        ones8 = const.tile([128, 4, 256], fp8)
        nc.vector.memset(ones8, WS)
        eps_sb = const.tile([128, 1], f32)
        nc.vector.memset(eps_sb, EPS)
        bqk_sb = const.tile([128, 16], f32)
        nc.sync.dma_start(out=bqk_sb[:], in_=bqk_d)
        # bias rows: 4KB DMA + on-chip broadcast (not a 512KB broadcast DMA)
        brow_o = const.tile([1, C], f32)
        nc.sync.dma_start(out=brow_o[:], in_=_bcast_ap(bout_d, p=1))
        brow_v = const.tile([1, C], f32)
        nc.sync.dma_start(out=brow_v[:], in_=_bcast_ap(bv32_d, p=1))
        bout_bc = const.tile([128, C], f32)
        nc.gpsimd.partition_broadcast(bout_bc[:], brow_o[:])
        bv_bc = const.tile([128, C], f32)
        if apply_affine:
            brow_g = const.tile([1, C], f32)
            nc.sync.dma_start(out=brow_g[:], in_=_bcast_ap(g_d, p=1))
            brow_b = const.tile([1, C], f32)
            nc.sync.dma_start(out=brow_b[:], in_=_bcast_ap(b_d, p=1))
            g_bc = const.tile([128, C], f32)
            nc.gpsimd.partition_broadcast(g_bc[:], brow_g[:])
            b_bc = const.tile([128, C], f32)
            nc.gpsimd.partition_broadcast(b_bc[:], brow_b[:])

        # Persistent weights (fp8, host-prescaled by 32, already in sbuf layout).
        # DMAs are emitted inside phase 1 (after the first x tiles) so the x
        # stream wins the early HBM bandwidth.
        w_pool = ctx.enter_context(tc.tile_pool(name="w8", bufs=1, side="left"))
        wqk8i = w_pool.tile([128, 4, 16, 256], fp8)
        wv8 = w_pool.tile([128, NG, C], fp8)
        wout8 = w_pool.tile([128, NG, C], fp8)

        xn_pool = ctx.enter_context(tc.tile_pool(name="xn", bufs=1, side="left"))
        xn = xn_pool.tile([128, NT, C], bf16)      # normalized x, natural [l, c]
        xnb_pool = ctx.enter_context(tc.tile_pool(name="xnb", bufs=1, side="left"))
        xnb = xnb_pool.tile([128, NT, C], bf16)    # xn + b_out (residual operand)
        xnT_pool = ctx.enter_context(tc.tile_pool(name="xnT8", bufs=1, side="left"))
        xnT8 = xnT_pool.tile([128, NG, L], fp8)    # [c', g, l] (moving operand)
        xnTi_pool = ctx.enter_context(tc.tile_pool(name="xnT8i", bufs=1, side="left"))
        xnT8i = xnTi_pool.tile([128, 4, NT, 256], fp8)  # interleaved stationary
        qT_pool = ctx.enter_context(tc.tile_pool(name="qT", bufs=1, side="left"))
        qT = qT_pool.tile([128, H, NG, 128], fp8)   # [c', h, g_q, l_r]
        kT_pool = ctx.enter_context(tc.tile_pool(name="kT", bufs=1, side="left"))
        kT = kT_pool.tile([128, NG, L], fp8)        # [c', g_k, l]
        v_pool = ctx.enter_context(tc.tile_pool(name="v8i", bufs=1, side="left"))
        v8i = v_pool.tile([128, NT, 4, 256], fp8)   # interleaved 32*v stationary
        attnT_pool = ctx.enter_context(tc.tile_pool(name="attnT8i", bufs=1,
                                                    side="left"))
        attnT8i = attnT_pool.tile([128, H, 4, 256], fp8)  # interleaved stationary

        # ---- Phase 1: LayerNorm + transpose (+ copies), per l-tile ----
        with tc.tile_pool(name="xin", bufs=4, side="right") as xin, \
             tc.tile_pool(name="lnst", bufs=4, side="right") as lnst, \
             tc.tile_pool(name="lntmp", bufs=3, side="right") as lntmp, \
             tc.tile_pool(name="tr_ps", bufs=2, space="PSUM") as tr_ps:
            for t in range(NT):
                xt = xin.tile([128, C], f32)
                stats = lnst.tile([128, 2, 6], f32)
                for j in range(2):
                    nc.sync.dma_start(
                        out=xt[:, 512 * j:512 * (j + 1)],
                        in_=x_d[128 * t:128 * (t + 1), 512 * j:512 * (j + 1)])
                    nc.vector.bn_stats(out=stats[:, j, :],
                                       in_=xt[:, 512 * j:512 * (j + 1)])
                if t == 2:
                    # weight DMAs queue behind the first three x tiles
                    nc.gpsimd.dma_start(out=wqk8i[:].bitcast(u8), in_=wqk8_d)
                    nc.gpsimd.dma_start(out=wv8[:].bitcast(u8), in_=wv8_d)
                    nc.gpsimd.dma_start(out=wout8[:].bitcast(u8), in_=wout8_d)
                mv = lnst.tile([128, 2], f32)
                nc.vector.bn_aggr(out=mv[:], in_=stats[:])
                sq = lnst.tile([128, 1], f32)
                nc.scalar.activation(out=sq[:], in_=mv[:, 1:2], func=AF.Sqrt,
                                     bias=eps_sb[:], scale=1.0)
                rstd = lnst.tile([128, 1], f32)
                nc.vector.reciprocal(out=rstd[:], in_=sq[:])
                nmr = lnst.tile([128, 1], f32)
                nc.vector.tensor_scalar(nmr[:], mv[:, 0:1], rstd[:], -1.0,
                                        ALU.mult, ALU.mult)
                if apply_affine:
                    zt = lntmp.tile([128, C], f32)
                    nc.scalar.activation(out=zt[:], in_=xt[:], func=AF.Identity,
                                         bias=nmr[:], scale=rstd[:])
                    zg = lntmp.tile([128, C], f32)
                    nc.vector.tensor_tensor(out=zg[:], in0=zt[:], in1=g_bc[:],
                                            op=ALU.mult)
                    nc.vector.tensor_tensor(out=xn[:, t, :], in0=zg[:], in1=b_bc[:],
                                            op=ALU.add)
                else:
                    nc.scalar.activation(out=xn[:, t, :], in_=xt[:], func=AF.Identity,
                                         bias=nmr[:], scale=rstd[:])
                # residual operand xnb = xn + b_out  (off critical path, Pool)
                nc.gpsimd.tensor_tensor(out=xnb[:, t, :], in0=xn[:, t, :],
                                        in1=bout_bc[:], op=ALU.add)

                # transpose tile t: xn[l,c] blocks -> psum [c', g, l_r], one bank
                ps_t = tr_ps.tile([128, NG, 128], bf16)
                for g in range(NG):
                    nc.tensor.transpose(ps_t[:, g, :],
                                        xn[:, t, 128 * g:128 * (g + 1)], ident[:])
                # natural copy (moving operand layout) on scalar engine
                nc.scalar.copy(out=xnT8[:, :, 128 * t:128 * (t + 1)], in_=ps_t[:])
                # interleaved stationary copy on DVE (psum -> sbuf)
                nc.vector.tensor_copy(
                    _ilv_dst(xnT8i[:], 256 * t, [[NT * 256, 4]]),
                    ps_t[:].rearrange("p (i s) l -> p i s l", i=4))

        # ------- Phase 2: Q, K projections (DoubleRowSwInterleave fp8) -------
        with tc.tile_pool(name="qk_ps", bufs=3, space="PSUM") as qk_ps:
            for co in range(16):
                psq = qk_ps.tile([128, L], f32)
                for i in range(4):
                    lhsT = wqk8i[:, i, co, :]
                    for j in range(2):
                        nc.tensor.matmul(
                            psq[:, 512 * j:512 * (j + 1)], lhsT,
                            xnT8[:, 2 * i:2 * i + 2, 512 * j:512 * (j + 1)],
                            start=(i == 0), stop=(i == 3), perf_mode=DRI)
                bias_col = bqk_sb[:, co:co + 1]
                if co < 8:
                    # q: dst [c', h, l_r] over h (l = 128h + l_r)
                    nc.vector.tensor_scalar(
                        qT[:, :, co, :],
                        psq[:].rearrange("p (h l) -> p h l", h=H),
                        WSI, bias_col, ALU.mult, ALU.add)
                else:
                    nc.scalar.activation(out=kT[:, co - 8, :], in_=psq[:],
                                         func=AF.Identity, bias=bias_col, scale=WSI)

        # ---- Phase 3: V projection (stationary = interleaved xnT) ----
        nc.gpsimd.partition_broadcast(bv_bc[:], brow_v[:])
        with tc.tile_pool(name="v_ps", bufs=2, space="PSUM") as v_ps:
            for t in range(NT):
                psv = v_ps.tile([128, C], f32)
                for i in range(4):
                    lhsT = xnT8i[:, i, t, :]
                    for j in range(2):
                        nc.tensor.matmul(
                            psv[:, 512 * j:512 * (j + 1)], lhsT,
                            wv8[:, 2 * i:2 * i + 2, 512 * j:512 * (j + 1)],
                            start=(i == 0), stop=(i == 3), perf_mode=DRI)
                # v8i = interleaved(psv + 32*bv)  (= 32*v), fp8
                nc.vector.tensor_tensor(
                    out=_ilv_dst(v8i[:], 1024 * t, [[256, 4]]),
                    in0=psv[:].rearrange("p (i s c) -> p i s c", i=4, s=2),
                    in1=bv_bc[:].rearrange("p (i s c) -> p i s c", i=4, s=2),
                    op=ALU.add)

        # ---------------- Phase 4: attention + out-proj, per head ----------------
        with tc.tile_pool(name="pt", bufs=3, side="right") as pt_pool, \
             tc.tile_pool(name="rb", bufs=3, side="right") as rb_pool, \
             tc.tile_pool(name="recip", bufs=2, side="right") as recip_pool, \
             tc.tile_pool(name="otile", bufs=2, side="right") as ot_pool, \
             tc.tile_pool(name="s_ps", bufs=2, space="PSUM") as s_ps, \
             tc.tile_pool(name="sum_ps", bufs=2, space="PSUM") as sum_ps, \
             tc.tile_pool(name="avo_ps", bufs=1, space="PSUM") as avo_ps:
            pend = []   # (h, pt, rb) awaiting attnV+outproj; emitted one head behind

            def emit_scores(h):
                pt = pt_pool.tile([128, NG, L], fp8, name=f"pt{h}", tag="pt")
                hs = slice(128 * h, 128 * (h + 1))
                sums = [sum_ps.tile([128, 512], f32, tag="ps_sum",
                                    name=f"ps_sum{h}_{j}") for j in range(2)]
                qrow = qT[:, h, :, :].rearrange("p g l -> p (g l)")

                def emit_sums(pair):
                    for j in range(2):
                        nc.tensor.matmul(sums[j][:],
                                         ones8[:, pair, :],
                                         pt[:, 2 * pair:2 * pair + 2,
                                            512 * j:512 * (j + 1)],
                                         start=(pair == 0), stop=(pair == 3),
                                         perf_mode=DRI)

                for gk in range(NG):
                    ps_s = s_ps.tile([128, L], f32, tag="ps_s")
                    for j in range(2):
                        nc.tensor.matmul(ps_s[:, 512 * j:512 * (j + 1)],
                                         kT[:, gk, hs],
                                         qrow[:, 512 * j:512 * (j + 1)],
                                         start=True, stop=True)
                    nc.scalar.activation(out=pt[:, gk, :], in_=ps_s[:], func=AF.Exp,
                                         bias=0.0, scale=S2)
                    if gk % 2 == 1:
                        emit_sums(gk // 2)
                # rb = 1/(32*sum(exp)) broadcast over partitions
                recip = recip_pool.tile([1, L], f32, tag="recip")
                rb = rb_pool.tile([128, L], f32, tag="rb")
                for j in range(2):
                    js = slice(512 * j, 512 * (j + 1))
                    nc.vector.reciprocal_approx_fast(out=recip[:, js],
                                                     in_=sums[j][0:1, :])
                    nc.gpsimd.partition_broadcast(rb[:, js], recip[:, js])
                pend.append((h, pt, rb))

            def emit_attnv_outproj():
                h, pt, rb = pend.pop(0)
                ps_av = avo_ps.tile([128, L], f32, tag="ps_avo")
                for i in range(4):
                    lhsT = v8i[:, h, i, :]
                    for j in range(2):
                        nc.tensor.matmul(ps_av[:, 512 * j:512 * (j + 1)], lhsT,
                                         pt[:, 2 * i:2 * i + 2, 512 * j:512 * (j + 1)],
                                         start=(i == 0), stop=(i == 3), perf_mode=DRI)
                # attnT = (32*sum(P v)) * 1/(32*sum(P)) = attn  (fp8, interleaved)
                nc.vector.tensor_tensor(
                    out=_ilv_dst(attnT8i[:], 1024 * h, [[256, 4]]),
                    in0=ps_av[:].rearrange("p (i s l) -> p i s l", i=4, s=2),
                    in1=rb[:].rearrange("p (i s l) -> p i s l", i=4, s=2),
                    op=ALU.mult)
                # out projection for m-tile = h (DRI fp8, wout prescaled x32)
                ps_o = avo_ps.tile([128, C], f32, tag="ps_avo")
                for i in range(4):
                    lhsT = attnT8i[:, h, i, :]
                    for j in range(2):
                        nc.tensor.matmul(
                            ps_o[:, 512 * j:512 * (j + 1)], lhsT,
                            wout8[:, 2 * i:2 * i + 2, 512 * j:512 * (j + 1)],
                            start=(i == 0), stop=(i == 3), perf_mode=DRI)
                t1 = ot_pool.tile([128, C], f32)
                nc.vector.tensor_scalar(t1[:], ps_o[:], WSI, None, ALU.mult)
                t2 = ot_pool.tile([128, C], f32)
                nc.vector.tensor_tensor(out=t2[:], in0=t1[:], in1=xnb[:, h, :],
                                        op=ALU.add)
                nc.sync.dma_start(out=out_d[128 * h:128 * (h + 1), :], in_=t2[:])

            for h in range(H):
                emit_scores(h)
                if pend and h > 0:
                    emit_attnv_outproj()
            while pend:
                emit_attnv_outproj()

    return nc


_CACHE = {}


def _build(apply_affine: bool):
    key = apply_affine
    if key not in _CACHE:
        nc = bacc.Bacc("TRN2", target_bir_lowering=False, debug=False)
        _emit(nc, apply_affine)
        nc.compile()
        _CACHE[key] = nc
    return _CACHE[key]


def _prep_maps(x, ln_g, ln_b, w_qkv, b_qkv, w_out, b_out):
    """Host-side prep: fp8 weight conversion + layouts. Returns (affine, in_maps)."""
    B = x.shape[0]
    apply_affine = not (np.all(ln_g == 1.0) and np.all(ln_b == 0.0))
    e4 = ml_dtypes.float8_e4m3fn

    # [c_in, c_out] -> [p, ki, c_out] with c_in = 128*ki + p  (moving layout)
    def lay(w):
        return np.ascontiguousarray(
            w.reshape(NG, 128, w.shape[1]).transpose(1, 0, 2))

    # QK weights: interleaved stationary layout
    # wqk8i[p, i, co, 2j+s] = 32*w_qkv[(2i+s)*128+p, 128*co + 127-j]
    Wr = (WS * w_qkv[:, :2 * C]).reshape(4, 2, 128, 16, 128)[:, :, :, :, ::-1]
    wqk8i = np.ascontiguousarray(
        Wr.transpose(2, 0, 3, 4, 1)).reshape(128, 4, 16, 256).astype(e4)

    wv8 = lay((WS * w_qkv[:, 2 * C:])).astype(e4).view(np.uint8)
    wout8 = lay((WS * w_out)).astype(e4).view(np.uint8)
    bqk_pre = np.ascontiguousarray(b_qkv[:2 * C].reshape(16, 128).T)
    bv32 = np.ascontiguousarray(WS * b_qkv[2 * C:])

    in_maps = []
    for c in range(B):
        m = {
            "x": np.ascontiguousarray(x[c]),
            "wqk8i": wqk8i.view(np.uint8),
            "wv8": wv8,
            "wout8": wout8,
            "b_qk": bqk_pre,
            "b_v32": bv32,
            "b_out": b_out,
        }
        if apply_affine:
            m["ln_g"] = ln_g
            m["ln_b"] = ln_b
        in_maps.append(m)
    return apply_affine, in_maps


def kernel(**inputs) -> np.ndarray:
    x = np.asarray(inputs["x"], np.float32)
    ln_g = np.asarray(inputs["ln_g"], np.float32)
    ln_b = np.asarray(inputs["ln_b"], np.float32)
    w_qkv = np.ascontiguousarray(np.asarray(inputs["w_qkv"], np.float32))
    b_qkv = np.asarray(inputs["b_qkv"], np.float32)
    w_out = np.ascontiguousarray(np.asarray(inputs["w_out"], np.float32))
    b_out = np.asarray(inputs["b_out"], np.float32)

    B = x.shape[0]
    assert x.shape == (B, L, C)
    apply_affine, in_maps = _prep_maps(x, ln_g, ln_b, w_qkv, b_qkv, w_out, b_out)
    nc = _build(apply_affine)
    res = bass_utils.run_bass_kernel_spmd(nc, in_maps, core_ids=list(range(B)))
    return np.stack([res.results[c]["out"] for c in range(B)]).astype(np.float32)


# revision 32
# speedup vs baseline: 1.0030x; 1.0030x over previous
"""Trainium2 Bass kernel for an AttentionBlock (LN -> QKV -> attn -> out-proj + residual).

Shapes (hardcoded per problem spec): B=8, L=1024, C=1024, H=8 heads.
The reference uses a raw row-major reshape (torch-style .view) of q/k/v from
[B, L, C] to [B*H, L, C/H]; with L=1024, C=1024, H=8 this makes each
"attention head" operate on a contiguous 128-sequence-row block of the
[L, C] matrix, reinterpreted as [1024, 128].

Sharding: pure data-parallel over batch, one batch element per NeuronCore
(8 cores). No collectives.

Perf design: the large contractions (QKV projection, attn*V, softmax sum,
out projection) run as fp8e4m3 matmuls in DoubleRowSwInterleave perf mode
(two 128-deep k-subtiles per instruction; ~2x bf16 FLOP rate). The
stationary operand must be software-interleaved: per partition the free
dim holds [A_127, B_127, ..., A_0, B_0] (A/B = the two k-subtiles'
weights per output column, columns reversed). Host weights arrive
pre-interleaved; device-produced stationaries (xnT, v, attnT) are written
in that layout via negative-stride APs in their epilogues. Weights are
pre-scaled by 32 so fp8 stays in normal range; the 1/32 is folded into
the bias / softmax-normalization epilogues. Scores stay bf16.

Phase order keeps the PE dense (DVFS ramps with utilization):
LN+transpose -> QK proj (interleaved xnT copies overlap) -> V proj ->
per-head [scores -> exp -> sums -> attnV -> out-proj -> residual].
"""

import math
from contextlib import ExitStack

import ml_dtypes
import numpy as np

import concourse.bass as bass
import concourse.bacc as bacc
import concourse.tile as tile
from concourse import mybir
from concourse import bass_utils
from concourse.masks import make_identity

L = 1024
C = 1024
H = 8          # heads; also number of 128-row l-tiles (head h <-> l-tile h)
CH = 128       # head dim
NT = 8         # l tiles (128 rows each)
NG = 8         # c groups (128 cols each)
EPS = 1e-5
S2 = 1.0 / math.sqrt(CH)   # combined q&k scale: (ch^-0.25)^2
WS = 32.0                  # host-side fp8 weight pre-scale
WSI = 1.0 / WS

f32 = mybir.dt.float32
bf16 = mybir.dt.bfloat16
fp8 = mybir.dt.float8e4
u8 = mybir.dt.uint8
AF = mybir.ActivationFunctionType
ALU = mybir.AluOpType
DRI = mybir.MatmulPerfMode.DoubleRowSwInterleave


def _bcast_ap(ap, p=128):
    """View a 1-D DRAM vector as [p, n] with a step-0 partition dim."""
    return bass.AP(tensor=ap.tensor, offset=ap.offset, ap=[[0, p]] + list(ap.ap))


def _ilv_dst(base, extra_offset, outer):
    """Interleaved stationary-layout destination AP: dims (*outer, s=2, j=128)
    with strides (*outer, +1, -2) from offset+254 - writes buf[2j+s] with the
    column index reversed, the layout DoubleRowSwInterleave ldweights wants."""
    return bass.AP(tensor=base.tensor, offset=base.offset + extra_offset + 254,
                   ap=[list(base.ap[0])] + outer + [[1, 2], [-2, 128]])


def _emit(nc, apply_affine: bool):
    x_d = nc.dram_tensor("x", [L, C], f32, kind="ExternalInput").ap()
    # fp8 weights travel as uint8 (XLA on TRN2 rejects fp8 parameter dtypes)
    wqk8_d = nc.dram_tensor("wqk8i", [128, 4, 16, 256], u8, kind="ExternalInput").ap()
    wv8_d = nc.dram_tensor("wv8", [128, NG, C], u8, kind="ExternalInput").ap()
    wout8_d = nc.dram_tensor("wout8", [128, NG, C], u8, kind="ExternalInput").ap()
    bqk_d = nc.dram_tensor("b_qk", [128, 16], f32, kind="ExternalInput").ap()
    bv32_d = nc.dram_tensor("b_v32", [C], f32, kind="ExternalInput").ap()
    bout_d = nc.dram_tensor("b_out", [C], f32, kind="ExternalInput").ap()
    if apply_affine:
        g_d = nc.dram_tensor("ln_g", [C], f32, kind="ExternalInput").ap()
        b_d = nc.dram_tensor("ln_b", [C], f32, kind="ExternalInput").ap()
    out_d = nc.dram_tensor("out", [L, C], f32, kind="ExternalOutput").ap()

    with nc.allow_low_precision(reason="bf16/fp8 compute by design"), \
         tile.TileContext(nc) as tc, ExitStack() as ctx:
        const = ctx.enter_context(tc.tile_pool(name="const", bufs=1, side="left"))
        ident = const.tile([128, 128], bf16)
        make_identity(nc, ident)
        # dual-fp8 ldweights wants the full 128 stationary columns -> 128
        # replicated ones columns (psum rows 1..127 hold sum copies, harmless)
        ones8 = const.tile([128, 4, 256], fp8)
        nc.vector.memset(ones8, WS)
        eps_sb = const.tile([128, 1], f32)
        nc.vector.memset(eps_sb, EPS)
        bqk_sb = const.tile([128, 16], f32)
        nc.sync.dma_start(out=bqk_sb[:], in_=bqk_d)
        # bias rows: 4KB DMA + on-chip broadcast (not a 512KB broadcast DMA)
        brow_o = const.tile([1, C], f32)
        nc.sync.dma_start(out=brow_o[:], in_=_bcast_ap(bout_d, p=1))
        brow_v = const.tile([1, C], f32)
        nc.sync.dma_start(out=brow_v[:], in_=_bcast_ap(bv32_d, p=1))
        bout_bc = const.tile([128, C], f32)
        nc.gpsimd.partition_broadcast(bout_bc[:], brow_o[:])
        bv_bc = const.tile([128, C], f32)
        if apply_affine:
            brow_g = const.tile([1, C], f32)
            nc.sync.dma_start(out=brow_g[:], in_=_bcast_ap(g_d, p=1))
            brow_b = const.tile([1, C], f32)
            nc.sync.dma_start(out=brow_b[:], in_=_bcast_ap(b_d, p=1))
            g_bc = const.tile([128, C], f32)
            nc.gpsimd.partition_broadcast(g_bc[:], brow_g[:])
            b_bc = const.tile([128, C], f32)
            nc.gpsimd.partition_broadcast(b_bc[:], brow_b[:])

        # Persistent weights (fp8, host-prescaled by 32, already in sbuf layout).
        # DMAs are emitted inside phase 1 (after the first x tiles) so the x
        # stream wins the early HBM bandwidth.
        w_pool = ctx.enter_context(tc.tile_pool(name="w8", bufs=1, side="left"))
        wqk8i = w_pool.tile([128, 4, 16, 256], fp8)
        wv8 = w_pool.tile([128, NG, C], fp8)
        wout8 = w_pool.tile([128, NG, C], fp8)

        xn_pool = ctx.enter_context(tc.tile_pool(name="xn", bufs=1, side="left"))
        xn = xn_pool.tile([128, NT, C], bf16)      # normalized x, natural [l, c]
        xnb_pool = ctx.enter_context(tc.tile_pool(name="xnb", bufs=1, side="left"))
        xnb = xnb_pool.tile([128, NT, C], bf16)    # xn + b_out (residual operand)
        xnT_pool = ctx.enter_context(tc.tile_pool(name="xnT8", bufs=1, side="left"))
        xnT8 = xnT_pool.tile([128, NG, L], fp8)    # [c', g, l] (moving operand)
        xnTi_pool = ctx.enter_context(tc.tile_pool(name="xnT8i", bufs=1, side="left"))
        xnT8i = xnTi_pool.tile([128, 4, NT, 256], fp8)  # interleaved stationary
        qT_pool = ctx.enter_context(tc.tile_pool(name="qT", bufs=1, side="left"))
        qT = qT_pool.tile([128, H, NG, 128], bf16)  # [c', h, g_q, l_r]
        kT_pool = ctx.enter_context(tc.tile_pool(name="kT", bufs=1, side="left"))
        kT = kT_pool.tile([128, NG, L], bf16)       # [c', g_k, l]
        v_pool = ctx.enter_context(tc.tile_pool(name="v8i", bufs=1, side="left"))
        v8i = v_pool.tile([128, NT, 4, 256], fp8)   # interleaved 32*v stationary
        attnT_pool = ctx.enter_context(tc.tile_pool(name="attnT8i", bufs=1,
                                                    side="left"))
        attnT8i = attnT_pool.tile([128, H, 4, 256], fp8)  # interleaved stationary

        # ---- Phase 1: LayerNorm + transpose (+ copies), per l-tile ----
        with tc.tile_pool(name="xin", bufs=4, side="right") as xin, \
             tc.tile_pool(name="lnst", bufs=4, side="right") as lnst, \
             tc.tile_pool(name="lntmp", bufs=3, side="right") as lntmp, \
             tc.tile_pool(name="tr_ps", bufs=2, space="PSUM") as tr_ps:
            for t in range(NT):
                xt = xin.tile([128, C], f32)
                stats = lnst.tile([128, 2, 6], f32)
                for j in range(2):
                    nc.sync.dma_start(
                        out=xt[:, 512 * j:512 * (j + 1)],
                        in_=x_d[128 * t:128 * (t + 1), 512 * j:512 * (j + 1)])
                    nc.vector.bn_stats(out=stats[:, j, :],
                                       in_=xt[:, 512 * j:512 * (j + 1)])
                if t == 2:
                    # weight DMAs queue behind the first three x tiles
                    nc.gpsimd.dma_start(out=wqk8i[:].bitcast(u8), in_=wqk8_d)
                    nc.gpsimd.dma_start(out=wv8[:].bitcast(u8), in_=wv8_d)
                    nc.gpsimd.dma_start(out=wout8[:].bitcast(u8), in_=wout8_d)
                mv = lnst.tile([128, 2], f32)
                nc.vector.bn_aggr(out=mv[:], in_=stats[:])
                sq = lnst.tile([128, 1], f32)
                nc.scalar.activation(out=sq[:], in_=mv[:, 1:2], func=AF.Sqrt,
                                     bias=eps_sb[:], scale=1.0)
                rstd = lnst.tile([128, 1], f32)
                nc.vector.reciprocal(out=rstd[:], in_=sq[:])
                nmr = lnst.tile([128, 1], f32)
                nc.vector.tensor_scalar(nmr[:], mv[:, 0:1], rstd[:], -1.0,
                                        ALU.mult, ALU.mult)
                if apply_affine:
                    zt = lntmp.tile([128, C], f32)
                    nc.scalar.activation(out=zt[:], in_=xt[:], func=AF.Identity,
                                         bias=nmr[:], scale=rstd[:])
                    zg = lntmp.tile([128, C], f32)
                    nc.vector.tensor_tensor(out=zg[:], in0=zt[:], in1=g_bc[:],
                                            op=ALU.mult)
                    nc.vector.tensor_tensor(out=xn[:, t, :], in0=zg[:], in1=b_bc[:],
                                            op=ALU.add)
                else:
                    nc.scalar.activation(out=xn[:, t, :], in_=xt[:], func=AF.Identity,
                                         bias=nmr[:], scale=rstd[:])
                # residual operand xnb = xn + b_out  (off critical path, Pool)
                nc.gpsimd.tensor_tensor(out=xnb[:, t, :], in0=xn[:, t, :],
                                        in1=bout_bc[:], op=ALU.add)

                # transpose tile t: xn[l,c] blocks -> psum [c', g, l_r], one bank
                ps_t = tr_ps.tile([128, NG, 128], bf16)
                for g in range(NG):
                    nc.tensor.transpose(ps_t[:, g, :],
                                        xn[:, t, 128 * g:128 * (g + 1)], ident[:])
                # natural copy (moving operand layout) on scalar engine
                nc.scalar.copy(out=xnT8[:, :, 128 * t:128 * (t + 1)], in_=ps_t[:])
                # interleaved stationary copy on DVE (psum -> sbuf)
                nc.vector.tensor_copy(
                    _ilv_dst(xnT8i[:], 256 * t, [[NT * 256, 4]]),
                    ps_t[:].rearrange("p (i s) l -> p i s l", i=4))

        # ------- Phase 2: Q, K projections (DoubleRowSwInterleave fp8) -------
        with tc.tile_pool(name="qk_ps", bufs=3, space="PSUM") as qk_ps:
            for co in range(16):
                psq = qk_ps.tile([128, L], f32)
                for i in range(4):
                    lhsT = wqk8i[:, i, co, :]
                    for j in range(2):
                        nc.tensor.matmul(
                            psq[:, 512 * j:512 * (j + 1)], lhsT,
                            xnT8[:, 2 * i:2 * i + 2, 512 * j:512 * (j + 1)],
                            start=(i == 0), stop=(i == 3), perf_mode=DRI)
                bias_col = bqk_sb[:, co:co + 1]
                if co < 8:
                    # q: dst [c', h, l_r] over h (l = 128h + l_r)
                    nc.vector.tensor_scalar(
                        qT[:, :, co, :],
                        psq[:].rearrange("p (h l) -> p h l", h=H),
                        WSI, bias_col, ALU.mult, ALU.add)
                else:
                    nc.scalar.activation(out=kT[:, co - 8, :], in_=psq[:],
                                         func=AF.Identity, bias=bias_col, scale=WSI)

        # ---- Phase 3: V projection (stationary = interleaved xnT) ----
        nc.gpsimd.partition_broadcast(bv_bc[:], brow_v[:])
        with tc.tile_pool(name="v_ps", bufs=2, space="PSUM") as v_ps:
            for t in range(NT):
                psv = v_ps.tile([128, C], f32)
                for i in range(4):
                    lhsT = xnT8i[:, i, t, :]
                    for j in range(2):
                        nc.tensor.matmul(
                            psv[:, 512 * j:512 * (j + 1)], lhsT,
                            wv8[:, 2 * i:2 * i + 2, 512 * j:512 * (j + 1)],
                            start=(i == 0), stop=(i == 3), perf_mode=DRI)
                # v8i = interleaved(psv + 32*bv)  (= 32*v), fp8
                nc.vector.tensor_tensor(
                    out=_ilv_dst(v8i[:], 1024 * t, [[256, 4]]),
                    in0=psv[:].rearrange("p (i s c) -> p i s c", i=4, s=2),
                    in1=bv_bc[:].rearrange("p (i s c) -> p i s c", i=4, s=2),
                    op=ALU.add)

        # ---------------- Phase 4: attention + out-proj, per head ----------------
        with tc.tile_pool(name="pt", bufs=2, side="right") as pt_pool, \
             tc.tile_pool(name="rb", bufs=3, side="right") as rb_pool, \
             tc.tile_pool(name="recip", bufs=2, side="right") as recip_pool, \
             tc.tile_pool(name="otile", bufs=2, side="right") as ot_pool, \
             tc.tile_pool(name="s_ps", bufs=2, space="PSUM") as s_ps, \
             tc.tile_pool(name="sum_ps", bufs=2, space="PSUM") as sum_ps, \
             tc.tile_pool(name="avo_ps", bufs=1, space="PSUM") as avo_ps:
            pend = []   # (h, pt, rb) awaiting attnV+outproj; emitted one head behind

            def emit_scores(h):
                pt = pt_pool.tile([128, NG, L], fp8, name=f"pt{h}", tag="pt")
                hs = slice(128 * h, 128 * (h + 1))
                sums = [sum_ps.tile([128, 512], f32, tag="ps_sum",
                                    name=f"ps_sum{h}_{j}") for j in range(2)]
                qrow = qT[:, h, :, :].rearrange("p g l -> p (g l)")

                def emit_sums(pair):
                    for j in range(2):
                        nc.tensor.matmul(sums[j][:],
                                         ones8[:, pair, :],
                                         pt[:, 2 * pair:2 * pair + 2,
                                            512 * j:512 * (j + 1)],
                                         start=(pair == 0), stop=(pair == 3),
                                         perf_mode=DRI)

                for gk in range(NG):
                    ps_s = s_ps.tile([128, L], f32, tag="ps_s")
                    for j in range(2):
                        nc.tensor.matmul(ps_s[:, 512 * j:512 * (j + 1)],
                                         kT[:, gk, hs],
                                         qrow[:, 512 * j:512 * (j + 1)],
                                         start=True, stop=True)
                    nc.scalar.activation(out=pt[:, gk, :], in_=ps_s[:], func=AF.Exp,
                                         bias=0.0, scale=S2)
                    if gk % 2 == 1:
                        emit_sums(gk // 2)
                # rb = 1/(32*sum(exp)) broadcast over partitions
                recip = recip_pool.tile([1, L], f32, tag="recip")
                rb = rb_pool.tile([128, L], f32, tag="rb")
                for j in range(2):
                    js = slice(512 * j, 512 * (j + 1))
                    nc.vector.reciprocal_approx_fast(out=recip[:, js],
                                                     in_=sums[j][0:1, :])
                    nc.gpsimd.partition_broadcast(rb[:, js], recip[:, js])
                pend.append((h, pt, rb))

            def emit_attnv_outproj():
                h, pt, rb = pend.pop(0)
                ps_av = avo_ps.tile([128, L], f32, tag="ps_avo")
                for i in range(4):
                    lhsT = v8i[:, h, i, :]
                    for j in range(2):
                        nc.tensor.matmul(ps_av[:, 512 * j:512 * (j + 1)], lhsT,
                                         pt[:, 2 * i:2 * i + 2, 512 * j:512 * (j + 1)],
                                         start=(i == 0), stop=(i == 3), perf_mode=DRI)
                # attnT = (32*sum(P v)) * 1/(32*sum(P)) = attn  (fp8, interleaved)
                nc.vector.tensor_tensor(
                    out=_ilv_dst(attnT8i[:], 1024 * h, [[256, 4]]),
                    in0=ps_av[:].rearrange("p (i s l) -> p i s l", i=4, s=2),
                    in1=rb[:].rearrange("p (i s l) -> p i s l", i=4, s=2),
                    op=ALU.mult)
                # out projection for m-tile = h (DRI fp8, wout prescaled x32)
                ps_o = avo_ps.tile([128, C], f32, tag="ps_avo")
                for i in range(4):
                    lhsT = attnT8i[:, h, i, :]
                    for j in range(2):
                        nc.tensor.matmul(
                            ps_o[:, 512 * j:512 * (j + 1)], lhsT,
                            wout8[:, 2 * i:2 * i + 2, 512 * j:512 * (j + 1)],
                            start=(i == 0), stop=(i == 3), perf_mode=DRI)
                t1 = ot_pool.tile([128, C], f32)
                t2 = ot_pool.tile([128, C], f32)
                if h < H - 1:
                    nc.vector.tensor_scalar(t1[:], ps_o[:], WSI, None, ALU.mult)
                    nc.vector.tensor_tensor(out=t2[:], in0=t1[:],
                                            in1=xnb[:, h, :], op=ALU.add)
                    nc.sync.dma_start(out=out_d[128 * h:128 * (h + 1), :],
                                      in_=t2[:])
                else:
                    # last head: halve the serial epilogue->DMA tail
                    for j in range(2):
                        js = slice(512 * j, 512 * (j + 1))
                        nc.vector.tensor_scalar(t1[:, js], ps_o[:, js], WSI,
                                                None, ALU.mult)
                        nc.vector.tensor_tensor(out=t2[:, js], in0=t1[:, js],
                                                in1=xnb[:, h, js], op=ALU.add)
                        nc.sync.dma_start(out=out_d[128 * h:128 * (h + 1), js],
                                          in_=t2[:, js])

            for h in range(H):
                emit_scores(h)
                if pend and h > 0:
                    emit_attnv_outproj()
            while pend:
                emit_attnv_outproj()

    return nc


_CACHE = {}


def _build(apply_affine: bool):
    key = apply_affine
    if key not in _CACHE:
        nc = bacc.Bacc("TRN2", target_bir_lowering=False, debug=False)
        _emit(nc, apply_affine)
        nc.compile()
        _CACHE[key] = nc
    return _CACHE[key]


def _prep_maps(x, ln_g, ln_b, w_qkv, b_qkv, w_out, b_out):
    """Host-side prep: fp8 weight conversion + layouts. Returns (affine, in_maps)."""
    B = x.shape[0]
    apply_affine = not (np.all(ln_g == 1.0) and np.all(ln_b == 0.0))
    e4 = ml_dtypes.float8_e4m3fn

    # [c_in, c_out] -> [p, ki, c_out] with c_in = 128*ki + p  (moving layout)
    def lay(w):
        return np.ascontiguousarray(
            w.reshape(NG, 128, w.shape[1]).transpose(1, 0, 2))

    # QK weights: interleaved stationary layout
    # wqk8i[p, i, co, 2j+s] = 32*w_qkv[(2i+s)*128+p, 128*co + 127-j]
    Wr = (WS * w_qkv[:, :2 * C]).reshape(4, 2, 128, 16, 128)[:, :, :, :, ::-1]
    wqk8i = np.ascontiguousarray(
        Wr.transpose(2, 0, 3, 4, 1)).reshape(128, 4, 16, 256).astype(e4)

    wv8 = lay((WS * w_qkv[:, 2 * C:])).astype(e4).view(np.uint8)
    wout8 = lay((WS * w_out)).astype(e4).view(np.uint8)
    bqk_pre = np.ascontiguousarray(b_qkv[:2 * C].reshape(16, 128).T)
    bv32 = np.ascontiguousarray(WS * b_qkv[2 * C:])

    in_maps = []
    for c in range(B):
        m = {
            "x": np.ascontiguousarray(x[c]),
            "wqk8i": wqk8i.view(np.uint8),
            "wv8": wv8,
            "wout8": wout8,
            "b_qk": bqk_pre,
            "b_v32": bv32,
            "b_out": b_out,
        }
        if apply_affine:
            m["ln_g"] = ln_g
            m["ln_b"] = ln_b
        in_maps.append(m)
    return apply_affine, in_maps


def kernel(**inputs) -> np.ndarray:
    x = np.asarray(inputs["x"], np.float32)
    ln_g = np.asarray(inputs["ln_g"], np.float32)
    ln_b = np.asarray(inputs["ln_b"], np.float32)
    w_qkv = np.ascontiguousarray(np.asarray(inputs["w_qkv"], np.float32))
    b_qkv = np.asarray(inputs["b_qkv"], np.float32)
    w_out = np.ascontiguousarray(np.asarray(inputs["w_out"], np.float32))
    b_out = np.asarray(inputs["b_out"], np.float32)

    B = x.shape[0]
    assert x.shape == (B, L, C)
    apply_affine, in_maps = _prep_maps(x, ln_g, ln_b, w_qkv, b_qkv, w_out, b_out)
    nc = _build(apply_affine)
    res = bass_utils.run_bass_kernel_spmd(nc, in_maps, core_ids=list(range(B)))
    return np.stack([res.results[c]["out"] for c in range(B)]).astype(np.float32)
